# revision 23
# baseline (speedup 1.0000x reference)
"""Trainium2 Bass kernel for nn_ActorLayerGAT (GATv2 message passing).

Contract: kernel(**inputs) takes the FULL unsharded inputs (as produced by
setup_inputs) and returns (h_actor [B, D] fp32, alpha0 [E, H] fp32), matching
the reference. Internally: pure data parallel over 8 NeuronCores (batch B
sharded), all weights replicated.

Design (per core, feature-major layout [feature, batch] on-chip):
  - inputs are resharded host-side to feature-major [D, B/8] so SBUF tiles
    load with contiguous 1KB partition lines and no on-device transposes
  - all matmuls run as float32r (tf32-like, 1 cycle/row) on the PE:
      ctx = h @ Wc (+bc);  xl_n = nodes_n @ Wl;  xr_n = nodes_n @ Wr (+bl+br)
    with nodes_n = x_n + ctx computed in-place by DVE scalar_tensor_tensor
  - edge stage in bf16 (DVE 2x mode): u_e = xl[src]+xr[dst]; t = LeakyReLU(u)
    on ACT; per-(edge,head) logit dot on PE with att column as stationary
  - softmax over incoming edges without max-subtraction (logits are tiny:
    |logits| < ~1, exp is safe); denominator / gather / src-sum all via tiny
    mask matmuls on PE (masks built host-side from edge_index)
  - aggregation pooled = 0.2 * sum_e alpha_e * xl[src_e] (+ bias_out + bl)
    via PE rank-1 broadcasts of the per-node alpha sums and DVE MACs
  - LayerNorm feature-major: sums via PE ones-dots; apply via rank-1
    A = gamma (x) rstd, B = beta (x) 1 - gamma (x) (mu*rstd) broadcast tiles
"""

import sys

sys.path.insert(0, "/opt/trn_rl_repo")

import numpy as np
import ml_dtypes

import concourse.bacc as bacc
import concourse.bass as bass
import concourse.tile as tile
from concourse import mybir
from concourse.bass_utils import run_bass_kernel_spmd

B, D, H, C, NN, E = 32768, 512, 4, 128, 5, 10
N_CORES = 8
BC = B // N_CORES            # 4096 batch rows per core
NB = 256                     # batch tile (free dim); fp32r needs >= 256
NT = BC // NB                # 16 batch tiles per core
NCH = D // 128               # 4 feature chunks (also = heads, C == 128)
R = E * H                    # 40 (edge, head) rows
G20 = NN * H                 # 20 (node, head) rows
SLOPE = 0.2
EPS = 1e-5

F32 = mybir.dt.float32
F32R = mybir.dt.float32r
BF16 = mybir.dt.bfloat16

IN_NAMES = ["striker_identity", "striker_state", "bowler_identity",
            "bowler_state", "partnership"]


def _build(edges, need_xr_bias, need_bc=True):
    """Build the per-core SPMD Bass module. `edges` is a tuple of (src, dst)
    pairs (compile-time topology); `need_xr_bias` adds rank-1 bias matmuls
    into the xr products when bl+br != 0."""
    nc = bacc.Bacc("TRN2", target_bir_lowering=False, debug=False)

    xt = [nc.dram_tensor(f"x{i}", [D, BC], F32R, kind="ExternalInput")
          for i in range(NN)]
    ht = nc.dram_tensor("ht", [D, BC], F32R, kind="ExternalInput")
    Wc = nc.dram_tensor("Wc", [D, D], F32R, kind="ExternalInput")
    Wl = nc.dram_tensor("Wl", [D, D], F32R, kind="ExternalInput")
    Wr = nc.dram_tensor("Wr", [D, D], F32R, kind="ExternalInput")
    # bias/affine vectors, chunked [128, 4]: column c = values for features
    # c*128..(c+1)*128
    bc_d = nc.dram_tensor("bc", [128, NCH], F32, kind="ExternalInput")
    blbr_d = nc.dram_tensor("blbr", [1, D], F32R, kind="ExternalInput")
    bobl_d = nc.dram_tensor("bobl", [128, NCH], F32, kind="ExternalInput")
    gb_d = nc.dram_tensor("gb", [2, D], F32R, kind="ExternalInput")  # beta;gamma
    att_d = nc.dram_tensor("att4", [C, NCH * H], BF16, kind="ExternalInput")
    mde_d = nc.dram_tensor("mde", [H, E * G20], F32R, kind="ExternalInput")
    gge_d = nc.dram_tensor("gge", [G20, E * H], F32R, kind="ExternalInput")
    mse_d = nc.dram_tensor("mse", [H, E * G20], F32R, kind="ExternalInput")
    ones_r_d = nc.dram_tensor("ones_r", [1, 128], BF16, kind="ExternalInput")
    ones_c_d = nc.dram_tensor("ones_c", [128, 1], F32R, kind="ExternalInput")
    ones_n_d = nc.dram_tensor("ones_n", [1, NB], F32R, kind="ExternalInput")

    ho = nc.dram_tensor("ho", [D, BC], F32R, kind="ExternalOutput")
    alpha0 = nc.dram_tensor("alpha0", [H, E], F32R, kind="ExternalOutput")
    import os
    dbg = os.environ.get("K_DEBUG") == "1"
    if dbg:
        dbg_t = {nm: nc.dram_tensor(f"dbg_{nm}", shp, F32R, kind="ExternalOutput")
                 for nm, shp in [("nodes0", [128, NCH, NB]), ("xl0", [128, NCH, NB]),
                                 ("xr0", [128, NCH, NB]), ("u0", [128, NCH, NB]),
                                 ("t0", [128, NCH, NB]), ("lg", [H, E, NB]),
                                 ("rd", [G20, NB]), ("v", [G20, NB]),
                                 ("pooled", [128, NCH, NB]), ("rstd", [1, NB]),
                                 ("mu", [1, NB])]}

    # feature-major views of the [D, BC] dram tensors: (p, c, b)
    xt_v = [t.ap().rearrange("(c p) b -> p c b", p=128) for t in xt]
    ht_v = ht.ap().rearrange("(c p) b -> p c b", p=128)
    ho_v = ho.ap().rearrange("(c p) b -> p c b", p=128)
    # weight views: (k-partition, k-chunk, dout)
    Wc_v = Wc.ap().rearrange("(k p) d -> p k d", p=128)
    Wl_v = Wl.ap().rearrange("(k p) d -> p k d", p=128)
    Wr_v = Wr.ap().rearrange("(k p) d -> p k d", p=128)

    from contextlib import ExitStack
    with nc.allow_low_precision("bf16 edge/aggregation path is intentional"), \
         tile.TileContext(nc) as tc, ExitStack() as ctx:
        cst = ctx.enter_context(tc.tile_pool(name="cst", bufs=1))
        w_sb = {}
        for nm, view in (("wc", Wc_v), ("wl", Wl_v), ("wr", Wr_v)):
            t = cst.tile([128, NCH, D], F32R, name=f"{nm}_sb")
            nc.sync.dma_start(out=t[:], in_=view)
            w_sb[nm] = t
        bc_sb = cst.tile([128, NCH], F32, name="bc_sb")
        nc.sync.dma_start(out=bc_sb[:], in_=bc_d.ap())
        bobl_sb = cst.tile([128, NCH], F32, name="bobl_sb")
        nc.sync.dma_start(out=bobl_sb[:], in_=bobl_d.ap())
        blbr_sb = cst.tile([1, D], F32R, name="blbr_sb")
        nc.sync.dma_start(out=blbr_sb[:], in_=blbr_d.ap())
        gb_sb = cst.tile([2, D], F32R, name="gb_sb")
        nc.sync.dma_start(out=gb_sb[:], in_=gb_d.ap())
        gam_sb = cst.tile([1, D], F32R, name="gam_sb")
        nc.sync.dma_start(out=gam_sb[:], in_=gb_d.ap()[1:2, :])
        att_sb = cst.tile([C, NCH * H], BF16, name="att_sb")
        nc.sync.dma_start(out=att_sb[:], in_=att_d.ap())
        mde_sb = cst.tile([H, E, G20], F32R, name="mde_sb")
        nc.sync.dma_start(out=mde_sb[:], in_=mde_d.ap().rearrange("h (e g) -> h e g", e=E))
        gge_sb = cst.tile([G20, E, H], F32R, name="gge_sb")
        nc.sync.dma_start(out=gge_sb[:], in_=gge_d.ap().rearrange("g (e h) -> g e h", e=E))
        mse_sb = cst.tile([H, E, G20], F32R, name="mse_sb")
        nc.sync.dma_start(out=mse_sb[:], in_=mse_d.ap().rearrange("h (e g) -> h e g", e=E))
        ones_row = cst.tile([1, 128], BF16, name="ones_row")
        nc.sync.dma_start(out=ones_row[:], in_=ones_r_d.ap())
        ones_col = cst.tile([128, 1], F32R, name="ones_col")
        nc.sync.dma_start(out=ones_col[:], in_=ones_c_d.ap())
        onesnb = cst.tile([1, NB], F32R, name="onesnb")
        nc.sync.dma_start(out=onesnb[:], in_=ones_n_d.ap())

        xin = ctx.enter_context(tc.tile_pool(name="xin", bufs=2))
        xlr = ctx.enter_context(tc.tile_pool(name="xlr", bufs=2))
        edg = ctx.enter_context(tc.tile_pool(name="edg", bufs=2))
        sm = ctx.enter_context(tc.tile_pool(name="smp", bufs=2))
        agg = ctx.enter_context(tc.tile_pool(name="agg", bufs=2))
        rowp = ctx.enter_context(tc.tile_pool(name="rowp", bufs=1))
        hout = ctx.enter_context(tc.tile_pool(name="hout", bufs=2))
        pprod = ctx.enter_context(tc.tile_pool(name="pprod", bufs=2, space="PSUM"))
        psm = ctx.enter_context(tc.tile_pool(name="psm", bufs=1, space="PSUM"))
        plate = ctx.enter_context(tc.tile_pool(name="plate", bufs=3, space="PSUM"))

        for it in range(NT):
            b0 = it * NB
            # ---- load inputs (feature-major slices) ----
            xin_t = []
            for n in range(NN):
                xi = xin.tile([128, NCH, NB], F32R, name=f"xi{n}_{it}",
                              tag=f"xi{n}")
                nc.sync.dma_start(out=xi[:], in_=xt_v[n][:, :, b0:b0 + NB])
                xin_t.append(xi)
            hi = xin.tile([128, NCH, NB], F32R, name=f"hi_{it}", tag="hi")
            nc.sync.dma_start(out=hi[:], in_=ht_v[:, :, b0:b0 + NB])

            # ---- ctx = h @ Wc (+ bc via nodes STT) ----
            ctx_ps = pprod.tile([128, NCH, NB], F32, name=f"ctx_{it}",
                                tag="prod")
            for c in range(NCH):
                for k in range(NCH):
                    nc.tensor.matmul(
                        ctx_ps[:, c, :],
                        w_sb["wc"][:, k, c * 128:(c + 1) * 128],
                        hi[:, k, :],
                        start=(k == 0), stop=(k == NCH - 1))
            # nodes_n = (x_n + bc) + ctx, in place, still f32r
            for n in range(NN):
                if need_bc:
                    for c in range(NCH):
                        nc.vector.scalar_tensor_tensor(
                            out=xin_t[n][:, c, :], in0=xin_t[n][:, c, :],
                            scalar=bc_sb[:, c:c + 1], in1=ctx_ps[:, c, :],
                            op0=mybir.AluOpType.add, op1=mybir.AluOpType.add)
                else:
                    nc.vector.tensor_add(xin_t[n][:], xin_t[n][:], ctx_ps[:])

            if dbg and it == 0:
                nc.gpsimd.dma_start(out=dbg_t["nodes0"].ap(), in_=xin_t[0][:])
            # ---- products xl_n / xr_n -> bf16 SBUF ----
            xl_t, xr_t = [], []
            for n in range(NN):
                pl = pprod.tile([128, NCH, NB], F32, name=f"pl{n}_{it}",
                                tag="prod")
                for c in range(NCH):
                    for k in range(NCH):
                        nc.tensor.matmul(
                            pl[:, c, :],
                            w_sb["wl"][:, k, c * 128:(c + 1) * 128],
                            xin_t[n][:, k, :],
                            start=(k == 0), stop=(k == NCH - 1))
                xls = xlr.tile([128, NCH, NB], BF16, name=f"xl{n}_{it}",
                               tag="xl", bufs=10)
                nc.scalar.copy(xls[:], pl[:])
                xl_t.append(xls)

                pr = pprod.tile([128, NCH, NB], F32, name=f"pr{n}_{it}",
                                tag="prod")
                for c in range(NCH):
                    nmm = NCH + (1 if need_xr_bias else 0)
                    for k in range(NCH):
                        nc.tensor.matmul(
                            pr[:, c, :],
                            w_sb["wr"][:, k, c * 128:(c + 1) * 128],
                            xin_t[n][:, k, :],
                            start=(k == 0), stop=(k == nmm - 1))
                    if need_xr_bias:
                        nc.tensor.matmul(
                            pr[:, c, :],
                            blbr_sb[:, c * 128:(c + 1) * 128],
                            onesnb[:],
                            start=False, stop=True)
                xrs = xlr.tile([128, NCH, NB], BF16, name=f"xr{n}_{it}",
                               tag="xr", bufs=8)
                nc.scalar.copy(xrs[:], pr[:])
                xr_t.append(xrs)

            if dbg and it == 0:
                nc.gpsimd.dma_start(out=dbg_t["xl0"].ap(), in_=xl_t[0][:])
                nc.gpsimd.dma_start(out=dbg_t["xr0"].ap(), in_=xr_t[0][:])
            # ---- edges: u = xl[s] + xr[d]; t = lrelu(u); logit dots ----
            # logits laid out head-major [H, E, NB] so every engine access
            # stays at partition base 0
            lg_sb = sm.tile([H, E, NB], F32R, name=f"lg_{it}", tag="lg")
            for e, (s, d) in enumerate(edges):
                ut = edg.tile([128, NCH, NB], BF16, name=f"u{e}_{it}", tag="u")
                ueng = nc.gpsimd if e in (0, 5) else nc.vector
                ueng.tensor_add(ut[:], xl_t[s][:], xr_t[d][:])
                tt = edg.tile([128, NCH, NB], BF16, name=f"t{e}_{it}", tag="t")
                nc.scalar.activation(tt[:], ut[:],
                                     mybir.ActivationFunctionType.Lrelu,
                                     alpha=SLOPE)
                # block-diagonal att: accumulate 4 chunk-matmuls into [H, NB];
                # row h only receives chunk h's contribution
                if dbg and it == 0 and e == 0:
                    nc.gpsimd.dma_start(out=dbg_t["u0"].ap(), in_=ut[:])
                    nc.gpsimd.dma_start(out=dbg_t["t0"].ap(), in_=tt[:])
                dps = plate.tile([H, NB], F32, name=f"dp{e}_{it}", tag="late")
                for c in range(NCH):
                    nc.tensor.matmul(dps[:], att_sb[:, c * H:(c + 1) * H],
                                     tt[:, c, :],
                                     start=(c == 0), stop=(c == NCH - 1))
                nc.vector.tensor_copy(lg_sb[:, e, :], dps[:])

            if dbg and it == 0:
                nc.gpsimd.dma_start(out=dbg_t["lg"].ap(), in_=lg_sb[:])
            # ---- softmax over incoming edges (no max-sub) ----
            # z = exp(logits) in place, then alpha = z * (1/den) in place
            nc.scalar.activation(lg_sb[:], lg_sb[:],
                                 mybir.ActivationFunctionType.Exp)
            den_ps = psm.tile([G20, NB], F32, name=f"den_{it}", tag="sm")
            for e in range(E):
                nc.tensor.matmul(den_ps[:], mde_sb[:, e, :], lg_sb[:, e, :],
                                 start=(e == 0), stop=(e == E - 1))
            rd_sb = sm.tile([G20, NB], F32R, name=f"rd_{it}", tag="rd", bufs=1)
            nc.vector.reciprocal(rd_sb[:], den_ps[:])
            for e in range(E):
                rg_ps = plate.tile([H, NB], F32, name=f"rg{e}_{it}",
                                   tag="late")
                nc.tensor.matmul(rg_ps[:], gge_sb[:, e, :], rd_sb[:],
                                 start=True, stop=True)
                nc.vector.tensor_mul(lg_sb[:, e, :], lg_sb[:, e, :], rg_ps[:])
            if it == 0:
                nc.sync.dma_start(out=alpha0.ap(), in_=lg_sb[:, :, 0])

            # ---- v = 0.2 * per-(src node, head) alpha sums ----
            v_ps = psm.tile([G20, NB], F32, name=f"v_{it}", tag="sm")
            for e in range(E):
                nc.tensor.matmul(v_ps[:], mse_sb[:, e, :], lg_sb[:, e, :],
                                 start=(e == 0), stop=(e == E - 1))
            v_sb = sm.tile([G20, NB], BF16, name=f"v_{it}s", tag="v", bufs=1)
            nc.scalar.copy(v_sb[:], v_ps[:])
            # flatten to one partition so each (node, head) row can be a
            # base-partition-0 matmul operand
            v_row = sm.tile([1, G20, NB], BF16, name=f"v_{it}r", tag="vr", bufs=2)
            nc.sync.dma_start(out=v_row[:], in_=v_sb[:])

            if dbg and it == 0:
                nc.gpsimd.dma_start(out=dbg_t["rd"].ap(), in_=rd_sb[:])
                nc.gpsimd.dma_start(out=dbg_t["v"].ap(), in_=v_sb[:])
            # ---- aggregation: pooled = sum_n vb(n) * xl_n (+ bobl) ----
            pooled = agg.tile([128, NCH, NB], F32R, name=f"po_{it}",
                              tag="pooled")
            for c in range(NCH):
                tmp_t = []
                for n in range(NN):
                    vb_sb = agg.tile([128, NB], BF16, name=f"vb{n}{c}_{it}",
                                     tag="vb", bufs=4)
                    nc.gpsimd.partition_broadcast(
                        vb_sb[:], v_row[:, n * H + c, :])
                    tm = agg.tile([128, NB], BF16, name=f"tm{n}{c}_{it}",
                                  tag="tmp", bufs=6)
                    nc.vector.tensor_mul(tm[:], xl_t[n][:, c, :], vb_sb[:])
                    tmp_t.append(tm)
                a01 = agg.tile([128, NB], BF16, name=f"a01{c}_{it}", tag="ta")
                nc.vector.tensor_add(a01[:], tmp_t[0][:], tmp_t[1][:])
                a23 = agg.tile([128, NB], BF16, name=f"a23{c}_{it}", tag="tb")
                nc.vector.tensor_add(a23[:], tmp_t[2][:], tmp_t[3][:])
                a03 = agg.tile([128, NB], BF16, name=f"a03{c}_{it}", tag="ta")
                nc.vector.tensor_add(a03[:], a01[:], a23[:])
                nc.vector.scalar_tensor_tensor(
                    out=pooled[:, c, :], in0=a03[:],
                    scalar=bobl_sb[:, c:c + 1], in1=tmp_t[4][:],
                    op0=mybir.AluOpType.add, op1=mybir.AluOpType.add)

            # ---- LayerNorm stats: SX, SQ via PE ones-dots ----
            if dbg and it == 0:
                nc.gpsimd.dma_start(out=dbg_t["pooled"].ap(), in_=pooled[:])
            sx_ps = plate.tile([1, NB], F32, name=f"sx_{it}", tag="late")
            sq_ps = plate.tile([1, NB], F32, name=f"sq2_{it}", tag="late")
            psq = agg.tile([128, NCH, NB], F32R, name=f"sq_{it}", tag="psq",
                           bufs=1)
            nc.scalar.square(psq[:], pooled[:])
            for c in range(NCH):
                nc.tensor.matmul(sx_ps[:], ones_col[:], pooled[:, c, :],
                                 start=(c == 0), stop=(c == NCH - 1))
            for c in range(NCH):
                nc.tensor.matmul(sq_ps[:], ones_col[:], psq[:, c, :],
                                 start=(c == 0), stop=(c == NCH - 1))
            mu_sb = rowp.tile([1, NB], F32R, name=f"mu_{it}", tag="mu")
            nc.vector.tensor_scalar_mul(mu_sb[:], sx_ps[:], 1.0 / D)
            m2_sb = rowp.tile([1, NB], F32R, name=f"m2_{it}", tag="m2")
            nc.vector.tensor_scalar_mul(m2_sb[:], sq_ps[:], 1.0 / D)
            mu2 = rowp.tile([1, NB], F32R, name=f"mu2_{it}", tag="mu2")
            nc.vector.tensor_mul(mu2[:], mu_sb[:], mu_sb[:])
            veps = rowp.tile([1, NB], F32R, name=f"ve_{it}", tag="ve")
            nc.vector.scalar_tensor_tensor(
                out=veps[:], in0=m2_sb[:], scalar=EPS, in1=mu2[:],
                op0=mybir.AluOpType.add, op1=mybir.AluOpType.subtract)
            rvar = rowp.tile([1, NB], F32R, name=f"rv_{it}", tag="rv")
            nc.vector.reciprocal(rvar[:], veps[:])
            rstd = rowp.tile([1, NB], F32R, name=f"rs_{it}", tag="rs")
            nc.scalar.sqrt(rstd[:], rvar[:])
            if dbg and it == 0:
                nc.gpsimd.dma_start(out=dbg_t["rstd"].ap(), in_=rstd[:])
                nc.gpsimd.dma_start(out=dbg_t["mu"].ap(), in_=mu_sb[:])
            # nmrs = -mu*rstd
            nmrs = rowp.tile([1, NB], F32R, name=f"nm_{it}", tag="nm")
            nc.vector.scalar_tensor_tensor(
                out=nmrs[:], in0=mu_sb[:], scalar=-1.0, in1=rstd[:],
                op0=mybir.AluOpType.mult, op1=mybir.AluOpType.mult)

            # ---- apply: h = pooled * (gamma x rstd) + (beta x 1 - gamma x mrs)
            h4 = hout.tile([128, NCH, NB], F32R, name=f"h_{it}", tag="h")
            for c in range(NCH):
                a_ps = plate.tile([128, NB], F32, name=f"A{c}_{it}",
                                  tag="late")
                nc.tensor.matmul(a_ps[:], gam_sb[:, c * 128:(c + 1) * 128],
                                 rstd[:], start=True, stop=True)
                b_ps = plate.tile([128, NB], F32, name=f"B{c}_{it}",
                                  tag="late")
                nc.tensor.matmul(b_ps[:], gb_sb[0:1, c * 128:(c + 1) * 128],
                                 onesnb[:], start=True, stop=False)
                nc.tensor.matmul(b_ps[:], gam_sb[:, c * 128:(c + 1) * 128],
                                 nmrs[:], start=False, stop=True)
                e1 = hout.tile([128, NB], F32R, name=f"e1{c}_{it}", tag="e1")
                nc.vector.tensor_mul(e1[:], pooled[:, c, :], a_ps[:])
                nc.vector.tensor_add(h4[:, c, :], e1[:], b_ps[:])
            nc.sync.dma_start(out=ho_v[:, :, b0:b0 + NB], in_=h4[:])

    nc.finalize()
    return nc


_KCACHE = {}


def _get_kernel(edges, flags):
    key = (edges, flags)
    if key not in _KCACHE:
        _KCACHE[key] = _build(edges, flags[0], flags[1])
    return _KCACHE[key]


def _prep_inputs(inputs):
    """Host-side prep: shard batch across cores, reshard big tensors to
    feature-major, build edge masks. Returns (edges, need_xr_bias, in_maps)."""
    f32 = np.float32
    ei = np.asarray(inputs["edge_index"])
    src, dst = ei[0], ei[1]
    edges = tuple((int(src[e]), int(dst[e])) for e in range(E))

    Wc = np.ascontiguousarray(np.asarray(inputs["Wc"], f32))
    Wl = np.ascontiguousarray(np.asarray(inputs["Wl"], f32))
    Wr = np.ascontiguousarray(np.asarray(inputs["Wr"], f32))
    bc = np.asarray(inputs["bc"], f32)
    bl = np.asarray(inputs["bl"], f32)
    br = np.asarray(inputs["br"], f32)
    bo = np.asarray(inputs["bias_out"], f32)
    gamma = np.asarray(inputs["gamma"], f32)
    beta = np.asarray(inputs["beta"], f32)
    att = np.asarray(inputs["att"], f32)

    blbr = bl + br
    need_xr_bias = bool(np.any(blbr != 0.0))
    need_bc = bool(np.any(bc != 0.0))

    bc_c = np.ascontiguousarray(bc.reshape(NCH, 128).T)
    bobl_c = np.ascontiguousarray((bo + bl).reshape(NCH, 128).T)
    gb = np.ascontiguousarray(np.stack([beta, gamma], 0))
    att4 = np.zeros((C, NCH * H), np.float32)
    for c in range(NCH):
        att4[:, c * H + c] = att[c]
    att4 = att4.astype(ml_dtypes.bfloat16)

    mde = np.zeros((H, E, G20), f32)
    mse = np.zeros((H, E, G20), f32)
    gge = np.zeros((G20, E, H), f32)
    for e in range(E):
        for h in range(H):
            mde[h, e, dst[e] * H + h] = 1.0
            mse[h, e, src[e] * H + h] = SLOPE  # folds the 1/N mean
            gge[dst[e] * H + h, e, h] = 1.0

    shared = {
        "Wc": Wc, "Wl": Wl, "Wr": Wr, "bc": bc_c,
        "blbr": blbr.reshape(1, D), "bobl": bobl_c, "gb": gb,
        "att4": np.ascontiguousarray(att4),
        "mde": mde.reshape(H, E * G20), "gge": gge.reshape(G20, E * H),
        "mse": mse.reshape(H, E * G20),
        "ones_r": np.ones((1, 128), ml_dtypes.bfloat16),
        "ones_c": np.ones((128, 1), f32),
        "ones_n": np.ones((1, NB), f32),
    }
    big = {f"x{i}": np.asarray(inputs[nm], f32)
           for i, nm in enumerate(IN_NAMES)}
    big["ht"] = np.asarray(inputs["h_state"], f32)

    in_maps = []
    for cidx in range(N_CORES):
        m = dict(shared)
        sl = slice(cidx * BC, (cidx + 1) * BC)
        for k, arr in big.items():
            m[k] = np.ascontiguousarray(arr[sl].T)
        in_maps.append(m)
    return edges, (need_xr_bias, need_bc), in_maps


def kernel(**inputs):
    edges, need_xr_bias, in_maps = _prep_inputs(inputs)
    nc = _get_kernel(edges, need_xr_bias)
    res = run_bass_kernel_spmd(nc, in_maps, core_ids=list(range(N_CORES)))
    h = np.concatenate(
        [np.ascontiguousarray(res.results[c]["ho"].T) for c in range(N_CORES)],
        axis=0)
    alpha0 = np.ascontiguousarray(res.results[0]["alpha0"].T).astype(np.float32)
    return h.astype(np.float32), alpha0


# revision 25
# speedup vs baseline: 3.1219x; 3.1219x over previous
"""Trainium2 Bass kernel for nn_ActorLayerGAT (GATv2 message passing).

Contract: kernel(**inputs) takes the FULL unsharded inputs (as produced by
setup_inputs) and returns (h_actor [B, D] fp32, alpha0 [E, H] fp32), matching
the reference. Internally: pure data parallel over 8 NeuronCores (batch B
sharded), all weights replicated.

Design (per core, feature-major layout [feature, batch] on-chip):
  - inputs are resharded host-side to feature-major [D, B/8] so SBUF tiles
    load with contiguous 1KB partition lines and no on-device transposes
  - all matmuls run as float32r (tf32-like, 1 cycle/row) on the PE:
      ctx = h @ Wc (+bc);  xl_n = nodes_n @ Wl;  xr_n = nodes_n @ Wr (+bl+br)
    with nodes_n = x_n + ctx computed in-place by DVE scalar_tensor_tensor
  - edge stage in bf16 (DVE 2x mode): u_e = xl[src]+xr[dst]; t = LeakyReLU(u)
    on ACT; per-(edge,head) logit dot on PE with att column as stationary
  - softmax over incoming edges without max-subtraction (logits are tiny:
    |logits| < ~1, exp is safe); denominator / gather / src-sum all via tiny
    mask matmuls on PE (masks built host-side from edge_index)
  - aggregation pooled = 0.2 * sum_e alpha_e * xl[src_e] (+ bias_out + bl)
    via PE rank-1 broadcasts of the per-node alpha sums and DVE MACs
  - LayerNorm feature-major: sums via PE ones-dots; apply via rank-1
    A = gamma (x) rstd, B = beta (x) 1 - gamma (x) (mu*rstd) broadcast tiles
"""

import sys

sys.path.insert(0, "/opt/trn_rl_repo")

import numpy as np
import ml_dtypes

import concourse.bacc as bacc
import concourse.bass as bass
import concourse.tile as tile
from concourse import mybir
from concourse.bass_utils import run_bass_kernel_spmd

B, D, H, C, NN, E = 32768, 512, 4, 128, 5, 10
N_CORES = 8
BC = B // N_CORES            # 4096 batch rows per core
NB = 256                     # batch tile (free dim); fp32r needs >= 256
NT = BC // NB                # 16 batch tiles per core
NCH = D // 128               # 4 feature chunks (also = heads, C == 128)
R = E * H                    # 40 (edge, head) rows
G20 = NN * H                 # 20 (node, head) rows
SLOPE = 0.2
EPS = 1e-5

F32 = mybir.dt.float32
F32R = mybir.dt.float32r
BF16 = mybir.dt.bfloat16

IN_NAMES = ["striker_identity", "striker_state", "bowler_identity",
            "bowler_state", "partnership"]


def _build(edges, need_xr_bias, need_bc=True):
    """Build the per-core SPMD Bass module. `edges` is a tuple of (src, dst)
    pairs (compile-time topology); `need_xr_bias` adds rank-1 bias matmuls
    into the xr products when bl+br != 0."""
    nc = bacc.Bacc("TRN2", target_bir_lowering=False, debug=False)

    xt = [nc.dram_tensor(f"x{i}", [D, BC], F32R, kind="ExternalInput")
          for i in range(NN)]
    ht = nc.dram_tensor("ht", [D, BC], F32R, kind="ExternalInput")
    Wc = nc.dram_tensor("Wc", [D, D], F32R, kind="ExternalInput")
    Wl = nc.dram_tensor("Wl", [D, D], F32R, kind="ExternalInput")
    Wr = nc.dram_tensor("Wr", [D, D], F32R, kind="ExternalInput")
    # bias/affine vectors, chunked [128, 4]: column c = values for features
    # c*128..(c+1)*128
    bc_d = nc.dram_tensor("bc", [128, NCH], F32, kind="ExternalInput")
    blbr_d = nc.dram_tensor("blbr", [1, D], F32R, kind="ExternalInput")
    bobl_d = nc.dram_tensor("bobl", [128, NCH], F32, kind="ExternalInput")
    gb_d = nc.dram_tensor("gb", [2, D], F32R, kind="ExternalInput")  # beta;gamma
    att_d = nc.dram_tensor("att4", [C, NCH * H], BF16, kind="ExternalInput")
    mde_d = nc.dram_tensor("mde", [H, E * G20], F32R, kind="ExternalInput")
    gge_d = nc.dram_tensor("gge", [G20, E * H], F32R, kind="ExternalInput")
    mse_d = nc.dram_tensor("mse", [H, E * G20], F32R, kind="ExternalInput")
    ones_r_d = nc.dram_tensor("ones_r", [1, 128], BF16, kind="ExternalInput")
    ones_c_d = nc.dram_tensor("ones_c", [128, 1], F32R, kind="ExternalInput")
    ones_n_d = nc.dram_tensor("ones_n", [1, NB], F32R, kind="ExternalInput")

    ho = nc.dram_tensor("ho", [D, BC], F32R, kind="ExternalOutput")
    alpha0 = nc.dram_tensor("alpha0", [H, E], F32R, kind="ExternalOutput")
    import os
    dbg = os.environ.get("K_DEBUG") == "1"
    if dbg:
        dbg_t = {nm: nc.dram_tensor(f"dbg_{nm}", shp, F32R, kind="ExternalOutput")
                 for nm, shp in [("nodes0", [128, NCH, NB]), ("xl0", [128, NCH, NB]),
                                 ("xr0", [128, NCH, NB]), ("u0", [128, NCH, NB]),
                                 ("t0", [128, NCH, NB]), ("lg", [H, E, NB]),
                                 ("rd", [G20, NB]), ("v", [G20, NB]),
                                 ("pooled", [128, NCH, NB]), ("rstd", [1, NB]),
                                 ("mu", [1, NB])]}

    # feature-major views of the [D, BC] dram tensors: (p, c, b)
    xt_v = [t.ap().rearrange("(c p) b -> p c b", p=128) for t in xt]
    ht_v = ht.ap().rearrange("(c p) b -> p c b", p=128)
    ho_v = ho.ap().rearrange("(c p) b -> p c b", p=128)
    # weight views: (k-partition, k-chunk, dout)
    Wc_v = Wc.ap().rearrange("(k p) d -> p k d", p=128)
    Wl_v = Wl.ap().rearrange("(k p) d -> p k d", p=128)
    Wr_v = Wr.ap().rearrange("(k p) d -> p k d", p=128)

    from contextlib import ExitStack
    with nc.allow_low_precision("bf16 edge/aggregation path is intentional"), \
         tile.TileContext(nc) as tc, ExitStack() as ctx:
        cst = ctx.enter_context(tc.tile_pool(name="cst", bufs=1))
        w_sb = {}
        for nm, view in (("wc", Wc_v), ("wl", Wl_v), ("wr", Wr_v)):
            t = cst.tile([128, NCH, D], F32R, name=f"{nm}_sb")
            nc.sync.dma_start(out=t[:], in_=view)
            w_sb[nm] = t
        bc_sb = cst.tile([128, NCH], F32, name="bc_sb")
        nc.sync.dma_start(out=bc_sb[:], in_=bc_d.ap())
        bobl_sb = cst.tile([128, NCH], F32, name="bobl_sb")
        nc.sync.dma_start(out=bobl_sb[:], in_=bobl_d.ap())
        blbr_sb = cst.tile([1, D], F32R, name="blbr_sb")
        nc.sync.dma_start(out=blbr_sb[:], in_=blbr_d.ap())
        gb_sb = cst.tile([2, D], F32R, name="gb_sb")
        nc.sync.dma_start(out=gb_sb[:], in_=gb_d.ap())
        gam_sb = cst.tile([1, D], F32R, name="gam_sb")
        nc.sync.dma_start(out=gam_sb[:], in_=gb_d.ap()[1:2, :])
        att_sb = cst.tile([C, NCH * H], BF16, name="att_sb")
        nc.sync.dma_start(out=att_sb[:], in_=att_d.ap())
        mde_sb = cst.tile([H, E, G20], F32R, name="mde_sb")
        nc.sync.dma_start(out=mde_sb[:], in_=mde_d.ap().rearrange("h (e g) -> h e g", e=E))
        gge_sb = cst.tile([G20, E, H], F32R, name="gge_sb")
        nc.sync.dma_start(out=gge_sb[:], in_=gge_d.ap().rearrange("g (e h) -> g e h", e=E))
        mse_sb = cst.tile([H, E, G20], F32R, name="mse_sb")
        nc.sync.dma_start(out=mse_sb[:], in_=mse_d.ap().rearrange("h (e g) -> h e g", e=E))
        ones_row = cst.tile([1, 128], BF16, name="ones_row")
        nc.sync.dma_start(out=ones_row[:], in_=ones_r_d.ap())
        ones_col = cst.tile([128, 1], F32R, name="ones_col")
        nc.sync.dma_start(out=ones_col[:], in_=ones_c_d.ap())
        onesnb = cst.tile([1, NB], F32R, name="onesnb")
        nc.sync.dma_start(out=onesnb[:], in_=ones_n_d.ap())

        xin = ctx.enter_context(tc.tile_pool(name="xin", bufs=2))
        xlr = ctx.enter_context(tc.tile_pool(name="xlr", bufs=2))  # xl/xr use explicit bufs
        edg = ctx.enter_context(tc.tile_pool(name="edg", bufs=2))
        sm = ctx.enter_context(tc.tile_pool(name="smp", bufs=2))
        agg = ctx.enter_context(tc.tile_pool(name="agg", bufs=2))
        rowp = ctx.enter_context(tc.tile_pool(name="rowp", bufs=1))
        hout = ctx.enter_context(tc.tile_pool(name="hout", bufs=2))
        pprod = ctx.enter_context(tc.tile_pool(name="pprod", bufs=2, space="PSUM"))
        psm = ctx.enter_context(tc.tile_pool(name="psm", bufs=1, space="PSUM"))
        plate = ctx.enter_context(tc.tile_pool(name="plate", bufs=3, space="PSUM"))

        for it in range(NT):
            b0 = it * NB
            # ---- load inputs (feature-major slices) ----
            xin_t = []
            for n in range(NN):
                xi = xin.tile([128, NCH, NB], F32R, name=f"xi{n}_{it}",
                              tag=f"xi{n}")
                nc.sync.dma_start(out=xi[:], in_=xt_v[n][:, :, b0:b0 + NB])
                xin_t.append(xi)
            hi = xin.tile([128, NCH, NB], F32R, name=f"hi_{it}", tag="hi", bufs=1)
            nc.sync.dma_start(out=hi[:], in_=ht_v[:, :, b0:b0 + NB])

            # ---- ctx = h @ Wc (+ bc via nodes STT) ----
            ctx_ps = pprod.tile([128, NCH, NB], F32, name=f"ctx_{it}",
                                tag="prod")
            for c in range(NCH):
                for k in range(NCH):
                    nc.tensor.matmul(
                        ctx_ps[:, c, :],
                        w_sb["wc"][:, k, c * 128:(c + 1) * 128],
                        hi[:, k, :],
                        start=(k == 0), stop=(k == NCH - 1))
            # nodes_n = (x_n + bc) + ctx, in place, still f32r
            for n in range(NN):
                if need_bc:
                    for c in range(NCH):
                        nc.vector.scalar_tensor_tensor(
                            out=xin_t[n][:, c, :], in0=xin_t[n][:, c, :],
                            scalar=bc_sb[:, c:c + 1], in1=ctx_ps[:, c, :],
                            op0=mybir.AluOpType.add, op1=mybir.AluOpType.add)
                else:
                    nc.vector.tensor_add(xin_t[n][:], xin_t[n][:], ctx_ps[:])

            if dbg and it == 0:
                nc.gpsimd.dma_start(out=dbg_t["nodes0"].ap(), in_=xin_t[0][:])
            # ---- products xl_n / xr_n -> bf16 SBUF ----
            xl_t, xr_t = [], []
            for n in range(NN):
                pl = pprod.tile([128, NCH, NB], F32, name=f"pl{n}_{it}",
                                tag="prod")
                for c in range(NCH):
                    for k in range(NCH):
                        nc.tensor.matmul(
                            pl[:, c, :],
                            w_sb["wl"][:, k, c * 128:(c + 1) * 128],
                            xin_t[n][:, k, :],
                            start=(k == 0), stop=(k == NCH - 1))
                xls = xlr.tile([128, NCH, NB], BF16, name=f"xl{n}_{it}",
                               tag="xl", bufs=10)
                nc.scalar.copy(xls[:], pl[:])
                xl_t.append(xls)

                pr = pprod.tile([128, NCH, NB], F32, name=f"pr{n}_{it}",
                                tag="prod")
                for c in range(NCH):
                    nmm = NCH + (1 if need_xr_bias else 0)
                    for k in range(NCH):
                        nc.tensor.matmul(
                            pr[:, c, :],
                            w_sb["wr"][:, k, c * 128:(c + 1) * 128],
                            xin_t[n][:, k, :],
                            start=(k == 0), stop=(k == nmm - 1))
                    if need_xr_bias:
                        nc.tensor.matmul(
                            pr[:, c, :],
                            blbr_sb[:, c * 128:(c + 1) * 128],
                            onesnb[:],
                            start=False, stop=True)
                xrs = xlr.tile([128, NCH, NB], BF16, name=f"xr{n}_{it}",
                               tag="xr", bufs=8)
                nc.scalar.copy(xrs[:], pr[:])
                xr_t.append(xrs)

            if dbg and it == 0:
                nc.gpsimd.dma_start(out=dbg_t["xl0"].ap(), in_=xl_t[0][:])
                nc.gpsimd.dma_start(out=dbg_t["xr0"].ap(), in_=xr_t[0][:])
            # ---- edges: u = xl[s] + xr[d]; t = lrelu(u); logit dots ----
            # logits laid out head-major [H, E, NB] so every engine access
            # stays at partition base 0
            lg_sb = sm.tile([H, E, NB], F32R, name=f"lg_{it}", tag="lg")
            for e, (s, d) in enumerate(edges):
                ut = edg.tile([128, NCH, NB], BF16, name=f"u{e}_{it}", tag="u")
                ueng = nc.gpsimd if e in (0, 5) else nc.vector
                ueng.tensor_add(ut[:], xl_t[s][:], xr_t[d][:])
                tt = edg.tile([128, NCH, NB], BF16, name=f"t{e}_{it}", tag="t")
                nc.scalar.activation(tt[:], ut[:],
                                     mybir.ActivationFunctionType.Lrelu,
                                     alpha=SLOPE)
                # block-diagonal att: accumulate 4 chunk-matmuls into [H, NB];
                # row h only receives chunk h's contribution
                if dbg and it == 0 and e == 0:
                    nc.gpsimd.dma_start(out=dbg_t["u0"].ap(), in_=ut[:])
                    nc.gpsimd.dma_start(out=dbg_t["t0"].ap(), in_=tt[:])
                dps = plate.tile([H, NB], F32, name=f"dp{e}_{it}", tag="late")
                for c in range(NCH):
                    nc.tensor.matmul(dps[:], att_sb[:, c * H:(c + 1) * H],
                                     tt[:, c, :],
                                     start=(c == 0), stop=(c == NCH - 1))
                nc.vector.tensor_copy(lg_sb[:, e, :], dps[:])

            if dbg and it == 0:
                nc.gpsimd.dma_start(out=dbg_t["lg"].ap(), in_=lg_sb[:])
            # ---- softmax over incoming edges (no max-sub) ----
            # z = exp(logits) in place, then alpha = z * (1/den) in place
            nc.scalar.activation(lg_sb[:], lg_sb[:],
                                 mybir.ActivationFunctionType.Exp)
            den_ps = psm.tile([G20, NB], F32, name=f"den_{it}", tag="sm")
            for e in range(E):
                nc.tensor.matmul(den_ps[:], mde_sb[:, e, :], lg_sb[:, e, :],
                                 start=(e == 0), stop=(e == E - 1))
            rd_sb = sm.tile([G20, NB], F32R, name=f"rd_{it}", tag="rd", bufs=1)
            nc.vector.reciprocal(rd_sb[:], den_ps[:])
            for e in range(E):
                rg_ps = plate.tile([H, NB], F32, name=f"rg{e}_{it}",
                                   tag="late")
                nc.tensor.matmul(rg_ps[:], gge_sb[:, e, :], rd_sb[:],
                                 start=True, stop=True)
                nc.vector.tensor_mul(lg_sb[:, e, :], lg_sb[:, e, :], rg_ps[:])
            if it == 0:
                nc.sync.dma_start(out=alpha0.ap(), in_=lg_sb[:, :, 0])

            # ---- v = 0.2 * per-(src node, head) alpha sums ----
            v_ps = psm.tile([G20, NB], F32, name=f"v_{it}", tag="sm")
            for e in range(E):
                nc.tensor.matmul(v_ps[:], mse_sb[:, e, :], lg_sb[:, e, :],
                                 start=(e == 0), stop=(e == E - 1))
            v_sb = sm.tile([G20, NB], BF16, name=f"v_{it}s", tag="v", bufs=1)
            nc.scalar.copy(v_sb[:], v_ps[:])
            # flatten to one partition so each (node, head) row can be a
            # base-partition-0 matmul operand
            v_row = sm.tile([1, G20, NB], BF16, name=f"v_{it}r", tag="vr", bufs=2)
            nc.sync.dma_start(out=v_row[:], in_=v_sb[:])

            if dbg and it == 0:
                nc.gpsimd.dma_start(out=dbg_t["rd"].ap(), in_=rd_sb[:])
                nc.gpsimd.dma_start(out=dbg_t["v"].ap(), in_=v_sb[:])
            # ---- aggregation: pooled = sum_n vb(n) * xl_n (+ bobl) ----
            pooled = agg.tile([128, NCH, NB], F32R, name=f"po_{it}",
                              tag="pooled")
            for c in range(NCH):
                tmp_t = []
                for n in range(NN):
                    vb_sb = agg.tile([128, NB], BF16, name=f"vb{n}{c}_{it}",
                                     tag="vb", bufs=4)
                    nc.gpsimd.partition_broadcast(
                        vb_sb[:], v_row[:, n * H + c, :])
                    tm = agg.tile([128, NB], BF16, name=f"tm{n}{c}_{it}",
                                  tag="tmp", bufs=6)
                    nc.vector.tensor_mul(tm[:], xl_t[n][:, c, :], vb_sb[:])
                    tmp_t.append(tm)
                a01 = agg.tile([128, NB], BF16, name=f"a01{c}_{it}", tag="ta")
                nc.vector.tensor_add(a01[:], tmp_t[0][:], tmp_t[1][:])
                a23 = agg.tile([128, NB], BF16, name=f"a23{c}_{it}", tag="tb")
                nc.vector.tensor_add(a23[:], tmp_t[2][:], tmp_t[3][:])
                a03 = agg.tile([128, NB], BF16, name=f"a03{c}_{it}", tag="ta")
                nc.vector.tensor_add(a03[:], a01[:], a23[:])
                nc.vector.scalar_tensor_tensor(
                    out=pooled[:, c, :], in0=a03[:],
                    scalar=bobl_sb[:, c:c + 1], in1=tmp_t[4][:],
                    op0=mybir.AluOpType.add, op1=mybir.AluOpType.add)

            # ---- LayerNorm stats: SX, SQ via PE ones-dots ----
            if dbg and it == 0:
                nc.gpsimd.dma_start(out=dbg_t["pooled"].ap(), in_=pooled[:])
            sx_ps = plate.tile([1, NB], F32, name=f"sx_{it}", tag="late")
            sq_ps = plate.tile([1, NB], F32, name=f"sq2_{it}", tag="late")
            psq = agg.tile([128, NCH, NB], F32R, name=f"sq_{it}", tag="psq",
                           bufs=1)
            nc.scalar.square(psq[:], pooled[:])
            for c in range(NCH):
                nc.tensor.matmul(sx_ps[:], ones_col[:], pooled[:, c, :],
                                 start=(c == 0), stop=(c == NCH - 1))
            for c in range(NCH):
                nc.tensor.matmul(sq_ps[:], ones_col[:], psq[:, c, :],
                                 start=(c == 0), stop=(c == NCH - 1))
            mu_sb = rowp.tile([1, NB], F32R, name=f"mu_{it}", tag="mu")
            nc.vector.tensor_scalar_mul(mu_sb[:], sx_ps[:], 1.0 / D)
            m2_sb = rowp.tile([1, NB], F32R, name=f"m2_{it}", tag="m2")
            nc.vector.tensor_scalar_mul(m2_sb[:], sq_ps[:], 1.0 / D)
            mu2 = rowp.tile([1, NB], F32R, name=f"mu2_{it}", tag="mu2")
            nc.vector.tensor_mul(mu2[:], mu_sb[:], mu_sb[:])
            veps = rowp.tile([1, NB], F32R, name=f"ve_{it}", tag="ve")
            nc.vector.scalar_tensor_tensor(
                out=veps[:], in0=m2_sb[:], scalar=EPS, in1=mu2[:],
                op0=mybir.AluOpType.add, op1=mybir.AluOpType.subtract)
            rvar = rowp.tile([1, NB], F32R, name=f"rv_{it}", tag="rv")
            nc.vector.reciprocal(rvar[:], veps[:])
            rstd = rowp.tile([1, NB], F32R, name=f"rs_{it}", tag="rs")
            nc.scalar.sqrt(rstd[:], rvar[:])
            if dbg and it == 0:
                nc.gpsimd.dma_start(out=dbg_t["rstd"].ap(), in_=rstd[:])
                nc.gpsimd.dma_start(out=dbg_t["mu"].ap(), in_=mu_sb[:])
            # nmrs = -mu*rstd
            nmrs = rowp.tile([1, NB], F32R, name=f"nm_{it}", tag="nm")
            nc.vector.scalar_tensor_tensor(
                out=nmrs[:], in0=mu_sb[:], scalar=-1.0, in1=rstd[:],
                op0=mybir.AluOpType.mult, op1=mybir.AluOpType.mult)

            # ---- apply: h = pooled * (gamma x rstd) + (beta x 1 - gamma x mrs)
            h4 = hout.tile([128, NCH, NB], F32R, name=f"h_{it}", tag="h")
            for c in range(NCH):
                a_ps = plate.tile([128, NB], F32, name=f"A{c}_{it}",
                                  tag="late")
                nc.tensor.matmul(a_ps[:], gam_sb[:, c * 128:(c + 1) * 128],
                                 rstd[:], start=True, stop=True)
                b_ps = plate.tile([128, NB], F32, name=f"B{c}_{it}",
                                  tag="late")
                nc.tensor.matmul(b_ps[:], gb_sb[0:1, c * 128:(c + 1) * 128],
                                 onesnb[:], start=True, stop=False)
                nc.tensor.matmul(b_ps[:], gam_sb[:, c * 128:(c + 1) * 128],
                                 nmrs[:], start=False, stop=True)
                e1 = hout.tile([128, NB], F32R, name=f"e1{c}_{it}", tag="e1")
                nc.vector.tensor_mul(e1[:], pooled[:, c, :], a_ps[:])
                nc.vector.tensor_add(h4[:, c, :], e1[:], b_ps[:])
            nc.sync.dma_start(out=ho_v[:, :, b0:b0 + NB], in_=h4[:])

    nc.finalize()
    return nc


_KCACHE = {}


def _get_kernel(edges, flags):
    key = (edges, flags)
    if key not in _KCACHE:
        _KCACHE[key] = _build(edges, flags[0], flags[1])
    return _KCACHE[key]


def _prep_inputs(inputs):
    """Host-side prep: shard batch across cores, reshard big tensors to
    feature-major, build edge masks. Returns (edges, need_xr_bias, in_maps)."""
    f32 = np.float32
    ei = np.asarray(inputs["edge_index"])
    src, dst = ei[0], ei[1]
    edges = tuple((int(src[e]), int(dst[e])) for e in range(E))

    Wc = np.ascontiguousarray(np.asarray(inputs["Wc"], f32))
    Wl = np.ascontiguousarray(np.asarray(inputs["Wl"], f32))
    Wr = np.ascontiguousarray(np.asarray(inputs["Wr"], f32))
    bc = np.asarray(inputs["bc"], f32)
    bl = np.asarray(inputs["bl"], f32)
    br = np.asarray(inputs["br"], f32)
    bo = np.asarray(inputs["bias_out"], f32)
    gamma = np.asarray(inputs["gamma"], f32)
    beta = np.asarray(inputs["beta"], f32)
    att = np.asarray(inputs["att"], f32)

    blbr = bl + br
    need_xr_bias = bool(np.any(blbr != 0.0))
    need_bc = bool(np.any(bc != 0.0))

    bc_c = np.ascontiguousarray(bc.reshape(NCH, 128).T)
    bobl_c = np.ascontiguousarray((bo + bl).reshape(NCH, 128).T)
    gb = np.ascontiguousarray(np.stack([beta, gamma], 0))
    att4 = np.zeros((C, NCH * H), np.float32)
    for c in range(NCH):
        att4[:, c * H + c] = att[c]
    att4 = att4.astype(ml_dtypes.bfloat16)

    mde = np.zeros((H, E, G20), f32)
    mse = np.zeros((H, E, G20), f32)
    gge = np.zeros((G20, E, H), f32)
    for e in range(E):
        for h in range(H):
            mde[h, e, dst[e] * H + h] = 1.0
            mse[h, e, src[e] * H + h] = SLOPE  # folds the 1/N mean
            gge[dst[e] * H + h, e, h] = 1.0

    shared = {
        "Wc": Wc, "Wl": Wl, "Wr": Wr, "bc": bc_c,
        "blbr": blbr.reshape(1, D), "bobl": bobl_c, "gb": gb,
        "att4": np.ascontiguousarray(att4),
        "mde": mde.reshape(H, E * G20), "gge": gge.reshape(G20, E * H),
        "mse": mse.reshape(H, E * G20),
        "ones_r": np.ones((1, 128), ml_dtypes.bfloat16),
        "ones_c": np.ones((128, 1), f32),
        "ones_n": np.ones((1, NB), f32),
    }
    big = {f"x{i}": np.asarray(inputs[nm], f32)
           for i, nm in enumerate(IN_NAMES)}
    big["ht"] = np.asarray(inputs["h_state"], f32)

    in_maps = []
    for cidx in range(N_CORES):
        m = dict(shared)
        sl = slice(cidx * BC, (cidx + 1) * BC)
        for k, arr in big.items():
            m[k] = np.ascontiguousarray(arr[sl].T)
        in_maps.append(m)
    return edges, (need_xr_bias, need_bc), in_maps


def kernel(**inputs):
    edges, need_xr_bias, in_maps = _prep_inputs(inputs)
    nc = _get_kernel(edges, need_xr_bias)
    res = run_bass_kernel_spmd(nc, in_maps, core_ids=list(range(N_CORES)))
    h = np.concatenate(
        [np.ascontiguousarray(res.results[c]["ho"].T) for c in range(N_CORES)],
        axis=0)
    alpha0 = np.ascontiguousarray(res.results[0]["alpha0"].T).astype(np.float32)
    return h.astype(np.float32), alpha0
